# revision 2
# baseline (speedup 1.0000x reference)
"""MultiHeadAttention Trainium2 kernel.

Full inputs -> full output. Sharding: 8 cores = (batch b in 0..3) x (query
half in 0..1). Each core computes attention for its 1024 query rows of batch
b over all 2048 keys of batch b (K/V projections duplicated across the pair
of cores sharing a batch), then applies the output projection for its rows.
Outputs are disjoint row-slices of (B, S, D); host assembly is a pure concat.

Per-core math (all matmuls in float32r on the PE, fp32 PSUM accumulate):
  phase A: transpose x chunks via PE-identity, project Q^T [d,s], K^T [d,s]
           (to DRAM), V [s,d] augmented with a ones column (to DRAM),
           rank-1 matmuls add bq/bk.
  phase B: per (head, 512-query-block): logits^T [sk,sq] = K_h^T.T @ Q_h^T,
           P = exp(0.125*logits + mask*(-1e9)) via ACT (per-partition bias),
           ctx^T/denom via [V_h | 1] stationary, reciprocal + ones-broadcast
           matmul to normalize, + bv.
  phase C: out = ctx @ wo + bo (ctx^T chunks are the stationary operand).
"""

import numpy as np

import concourse.bass as bass
import concourse.mybir as mybir
import concourse.tile as tile
from concourse import bacc
from concourse.bass_utils import run_bass_kernel_spmd

f32 = mybir.dt.float32
f32r = mybir.dt.float32r

B, S, D, H, DH = 4, 2048, 1024, 16, 64
SQ = S // 2          # query rows per core
N_CORES = 8
Exp = mybir.ActivationFunctionType.Exp
Identity = mybir.ActivationFunctionType.Identity


def _build():
    nc = bacc.Bacc(None, target_bir_lowering=False)

    xq = nc.dram_tensor("xq", [SQ, D], f32r, kind="ExternalInput")
    xk = nc.dram_tensor("xk", [S, D], f32r, kind="ExternalInput")
    xv = nc.dram_tensor("xv", [S, D], f32r, kind="ExternalInput")
    wq = nc.dram_tensor("wq", [D, D], f32r, kind="ExternalInput")
    wk = nc.dram_tensor("wk", [D, D], f32r, kind="ExternalInput")
    wv = nc.dram_tensor("wv", [D, D], f32r, kind="ExternalInput")
    wo = nc.dram_tensor("wo", [D, D], f32r, kind="ExternalInput")
    bq = nc.dram_tensor("bq", [1, D], f32r, kind="ExternalInput")
    bk = nc.dram_tensor("bk", [1, D], f32r, kind="ExternalInput")
    bo = nc.dram_tensor("bo", [1, D], f32r, kind="ExternalInput")
    bvT = nc.dram_tensor("bvT", [DH, H], f32, kind="ExternalInput")   # bv[h*64+p] at [p, h]
    mb = nc.dram_tensor("mb", [128, S // 128], f32, kind="ExternalInput")  # mask*-1e9 at [p, chunk]
    idn = nc.dram_tensor("idn", [128, 128], f32r, kind="ExternalInput")
    one = nc.dram_tensor("one", [128, 512], f32r, kind="ExternalInput")
    out = nc.dram_tensor("out", [SQ, D], f32, kind="ExternalOutput")

    with tile.TileContext(nc) as tc:
        _emit(nc, tc, xq, xk, xv, wq, wk, wv, wo, bq, bk, bo, bvT, mb, idn, one, out)
    nc.finalize()
    return nc


def _emit(nc, tc, xq, xk, xv, wq, wk, wv, wo, bq, bk, bo, bvT, mb, idn, one, out):
    from contextlib import ExitStack

    KC = D // 128            # 8 contraction chunks
    SKC = S // 128           # 16 key chunks
    with ExitStack() as ctx:
        consts = ctx.enter_context(tc.tile_pool(name="consts", bufs=1))
        wpool = ctx.enter_context(tc.tile_pool(name="wpool", bufs=1))
        xload = ctx.enter_context(tc.tile_pool(name="xload", bufs=2))
        xtp = ctx.enter_context(tc.tile_pool(name="xtp", bufs=2))
        qts = ctx.enter_context(tc.tile_pool(name="qts", bufs=2))
        kts = ctx.enter_context(tc.tile_pool(name="kts", bufs=2))
        vas = ctx.enter_context(tc.tile_pool(name="vas", bufs=2))
        ptp = ctx.enter_context(tc.tile_pool(name="ptp", bufs=1))
        stg = ctx.enter_context(tc.tile_pool(name="stg", bufs=3))
        nrm = ctx.enter_context(tc.tile_pool(name="nrm", bufs=2))
        cns = ctx.enter_context(tc.tile_pool(name="cns", bufs=2))
        psA = ctx.enter_context(tc.tile_pool(name="psA", bufs=3, space="PSUM"))
        psC = ctx.enter_context(tc.tile_pool(name="psC", bufs=2, space="PSUM"))
        psB = ctx.enter_context(tc.tile_pool(name="psB", bufs=1, space="PSUM"))
        psT = ctx.enter_context(tc.tile_pool(name="psT", bufs=2, space="PSUM"))
        dram = ctx.enter_context(tc.tile_pool(name="dram", bufs=1, space="DRAM"))

        ktd = dram.tile([D, S], f32r)               # K^T
        vad = dram.tile([SKC, 128, H, DH + 1], f32r)  # V augmented with ones col
        qtd = dram.tile([D, SQ], f32r)              # Q^T
        cnd = dram.tile([D, SQ], f32r)              # normalized ctx^T

        ident = consts.tile([128, 128], f32r)
        nc.sync.dma_start(ident, idn[:])
        ones = consts.tile([128, 512], f32r)
        nc.sync.dma_start(ones, one[:])
        bq_sb = consts.tile([1, D], f32r)
        nc.sync.dma_start(bq_sb, bq[:])
        bk_sb = consts.tile([1, D], f32r)
        nc.sync.dma_start(bk_sb, bk[:])
        bo_sb = consts.tile([1, D], f32r)
        nc.sync.dma_start(bo_sb, bo[:])
        bv_sb = consts.tile([DH, H], f32)
        nc.sync.dma_start(bv_sb, bvT[:])
        mb_sb = consts.tile([128, SKC], f32)
        nc.sync.dma_start(mb_sb, mb[:])

        # ---- helper: transpose a [128, 512] row-block of x into xT tile cols
        def transpose_block(xrow, kc0, xT, col0):
            """xrow [128 rows, 1024 d] -> xT[:, kc0..kc0+3, col0:col0+128] (4 chunks)."""
            pst = psT.tile([128, 512], f32r, tag="psT")
            for j in range(4):
                nc.tensor.matmul(
                    pst[:, j * 128:(j + 1) * 128],
                    lhsT=xrow[:, (kc0 + j) * 128:(kc0 + j + 1) * 128],
                    rhs=ident[:], is_transpose=True, start=True, stop=True,
                )
            nc.vector.tensor_copy(
                xT[:, kc0:kc0 + 4, col0:col0 + 128],
                pst.rearrange("p (j c) -> p j c", j=4),
            )

        # ================= phase A =================
        # --- Q^T -> qtd ---
        wq_sb = wpool.tile([128, KC, D], f32r, tag="w")
        nc.sync.dma_start(wq_sb, wq.rearrange("(ko p) d -> p ko d", p=128))
        for sqb in range(2):
            xT = xtp.tile([128, KC, 512], f32r, tag="xT")
            for st in range(4):
                xrow = xload.tile([128, D], f32r, tag="xr")
                nc.sync.dma_start(xrow, xq[(sqb * 4 + st) * 128:(sqb * 4 + st + 1) * 128, :])
                transpose_block(xrow, 0, xT, st * 128)
                transpose_block(xrow, 4, xT, st * 128)
            for dc in range(KC):
                ps = psA.tile([128, 512], f32, tag="psA")
                for kc in range(KC):
                    nc.tensor.matmul(ps, lhsT=wq_sb[:, kc, dc * 128:(dc + 1) * 128],
                                     rhs=xT[:, kc, :], start=(kc == 0), stop=False)
                nc.tensor.matmul(ps, lhsT=bq_sb[0:1, dc * 128:(dc + 1) * 128],
                                 rhs=ones[0:1, :], start=False, stop=True)
                st_t = stg.tile([128, 512], f32r, tag="stg")
                nc.vector.tensor_copy(st_t, ps)
                nc.sync.dma_start(qtd[dc * 128:(dc + 1) * 128, sqb * 512:(sqb + 1) * 512], st_t)

        # --- K^T -> ktd ---
        wk_sb = wpool.tile([128, KC, D], f32r, tag="w")
        nc.sync.dma_start(wk_sb, wk.rearrange("(ko p) d -> p ko d", p=128))
        for skb in range(4):
            xT = xtp.tile([128, KC, 512], f32r, tag="xT")
            for st in range(4):
                xrow = xload.tile([128, D], f32r, tag="xr")
                nc.sync.dma_start(xrow, xk[(skb * 4 + st) * 128:(skb * 4 + st + 1) * 128, :])
                transpose_block(xrow, 0, xT, st * 128)
                transpose_block(xrow, 4, xT, st * 128)
            for dc in range(KC):
                ps = psA.tile([128, 512], f32, tag="psA")
                for kc in range(KC):
                    nc.tensor.matmul(ps, lhsT=wk_sb[:, kc, dc * 128:(dc + 1) * 128],
                                     rhs=xT[:, kc, :], start=(kc == 0), stop=False)
                nc.tensor.matmul(ps, lhsT=bk_sb[0:1, dc * 128:(dc + 1) * 128],
                                 rhs=ones[0:1, :], start=False, stop=True)
                st_t = stg.tile([128, 512], f32r, tag="stg")
                nc.vector.tensor_copy(st_t, ps)
                nc.sync.dma_start(ktd[dc * 128:(dc + 1) * 128, skb * 512:(skb + 1) * 512], st_t)

        # --- V (augmented) -> vad ---
        wv_sb = wpool.tile([128, KC, D], f32r, tag="w")
        nc.sync.dma_start(wv_sb, wv.rearrange("(ko p) d -> p ko d", p=128))
        # ones column of the augmentation, written once
        for sc in range(SKC):
            nc.sync.dma_start(vad[sc, :, :, DH], ones[:, 0:H])
        for sc in range(SKC):
            xT = xtp.tile([128, KC, 512], f32r, tag="xT")
            xrow = xload.tile([128, D], f32r, tag="xr")
            nc.sync.dma_start(xrow, xv[sc * 128:(sc + 1) * 128, :])
            transpose_block(xrow, 0, xT, 0)
            transpose_block(xrow, 4, xT, 0)
            for dh2 in range(2):
                ps = psA.tile([128, 512], f32, tag="psA")
                for kc in range(KC):
                    nc.tensor.matmul(ps, lhsT=xT[:, kc, 0:128],
                                     rhs=wv_sb[:, kc, dh2 * 512:(dh2 + 1) * 512],
                                     start=(kc == 0), stop=(kc == KC - 1))
                st_t = stg.tile([128, 512], f32r, tag="stg")
                nc.vector.tensor_copy(st_t, ps)
                nc.sync.dma_start(
                    vad[sc, :, dh2 * 8:(dh2 + 1) * 8, 0:DH],
                    st_t.rearrange("p (h d) -> p h d", h=8),
                )

        # ================= phase B: attention =================
        for h in range(H):
            base = (h % 2) * 64
            kt = kts.tile([128, S], f32r, tag="kt")
            nc.sync.dma_start(kt[base:base + 64, :], ktd[h * 64:(h + 1) * 64, :])
            va = vas.tile([128, SKC, DH + 1], f32r, tag="va")
            nc.sync.dma_start(va, vad[:, :, h, :].rearrange("sc p c -> p sc c"))
            for sqb in range(2):
                qt = qts.tile([128, 512], f32r, tag="qt")
                nc.sync.dma_start(qt[base:base + 64, :],
                                  qtd[h * 64:(h + 1) * 64, sqb * 512:(sqb + 1) * 512])
                pt = ptp.tile([128, SKC, 512], f32r, tag="pt")
                for skc in range(SKC):
                    psl = psA.tile([128, 512], f32, tag="psA")
                    nc.tensor.matmul(psl, lhsT=kt[base:base + 64, skc * 128:(skc + 1) * 128],
                                     rhs=qt[base:base + 64, :], start=True, stop=True)
                    nc.scalar.activation(pt[:, skc, :], psl, Exp,
                                         bias=mb_sb[:, skc:skc + 1], scale=0.125)
                psc = psC.tile([128, 512], f32, tag="psC")
                for skc in range(SKC):
                    nc.tensor.matmul(psc[0:DH + 1, :], lhsT=va[:, skc, :],
                                     rhs=pt[:, skc, :],
                                     start=(skc == 0), stop=(skc == SKC - 1))
                # normalize: ctx rows 0..63 divided by denom row 64, then + bv
                recf = nrm.tile([65, 512], f32, tag="recf")
                nc.vector.reciprocal(recf[64:65, :], psc[64:65, :])
                rec = nrm.tile([65, 512], f32r, tag="rec")
                with nc.allow_low_precision(reason="softmax recip rounded to f32r"):
                    nc.vector.tensor_copy(rec[64:65, :], recf[64:65, :])
                psb = psB.tile([128, 512], f32, tag="psB")
                nc.tensor.matmul(psb[0:64, :], lhsT=ones[64:65, 0:64],
                                 rhs=rec[64:65, :], start=True, stop=True)
                bc = nrm.tile([64, 512], f32, tag="bc")
                nc.vector.tensor_copy(bc, psb[0:64, :])
                cn = cns.tile([64, 512], f32r, tag="cn")
                with nc.allow_low_precision(reason="ctx rounded to f32r for out proj"):
                    nc.vector.tensor_mul(out=cn, in0=psc[0:DH, :], in1=bc)
                nc.scalar.activation(cn, cn, Identity, bias=bv_sb[:, h:h + 1], scale=1.0)
                nc.sync.dma_start(cnd[h * 64:(h + 1) * 64, sqb * 512:(sqb + 1) * 512], cn)

        # ================= phase C: output projection =================
        wo_sb = wpool.tile([128, KC, D], f32r, tag="w")
        nc.sync.dma_start(wo_sb, wo.rearrange("(ko p) d -> p ko d", p=128))
        for st8 in range(8):
            cT = cns.tile([128, KC, 128], f32r, tag="cT")
            nc.sync.dma_start(cT, cnd[:, st8 * 128:(st8 + 1) * 128]
                              .rearrange("(ko p) s -> p ko s", p=128))
            for dh2 in range(2):
                ps = psA.tile([128, 512], f32, tag="psA")
                for kc in range(KC):
                    nc.tensor.matmul(ps, lhsT=cT[:, kc, :],
                                     rhs=wo_sb[:, kc, dh2 * 512:(dh2 + 1) * 512],
                                     start=(kc == 0), stop=False)
                nc.tensor.matmul(ps, lhsT=ones[0:1, 0:128],
                                 rhs=bo_sb[0:1, dh2 * 512:(dh2 + 1) * 512],
                                 start=False, stop=True)
                st_t = stg.tile([128, 512], f32, tag="ost")
                nc.vector.tensor_copy(st_t, ps)
                nc.sync.dma_start(out[st8 * 128:(st8 + 1) * 128,
                                      dh2 * 512:(dh2 + 1) * 512], st_t)


_NC_CACHE = None


def kernel(query, key, value, mask, wq, bq, wk, bk, wv, bv, wo, bo):
    global _NC_CACHE
    if _NC_CACHE is None:
        _NC_CACHE = _build()
    nc = _NC_CACHE

    query = np.ascontiguousarray(np.asarray(query, dtype=np.float32))
    key = np.ascontiguousarray(np.asarray(key, dtype=np.float32))
    value = np.ascontiguousarray(np.asarray(value, dtype=np.float32))
    mask = np.asarray(mask, dtype=np.float32)

    shared = {
        "wq": np.asarray(wq, np.float32), "wk": np.asarray(wk, np.float32),
        "wv": np.asarray(wv, np.float32), "wo": np.asarray(wo, np.float32),
        "bq": np.asarray(bq, np.float32).reshape(1, D),
        "bk": np.asarray(bk, np.float32).reshape(1, D),
        "bo": np.asarray(bo, np.float32).reshape(1, D),
        "bvT": np.ascontiguousarray(np.asarray(bv, np.float32).reshape(H, DH).T),
        "idn": np.eye(128, dtype=np.float32),
        "one": np.ones((128, 512), np.float32),
    }
    in_maps = []
    for core in range(N_CORES):
        b, half = divmod(core, 2)
        mbc = np.ascontiguousarray(
            (mask[b, 0, 0] * np.float32(-1e9)).reshape(S // 128, 128).T)
        in_maps.append({
            "xq": np.ascontiguousarray(query[b, half * SQ:(half + 1) * SQ]),
            "xk": key[b], "xv": value[b], "mb": mbc, **shared,
        })

    res = run_bass_kernel_spmd(nc, in_maps, core_ids=list(range(N_CORES)))
    full = np.empty((B, S, D), np.float32)
    for core in range(N_CORES):
        b, half = divmod(core, 2)
        full[b, half * SQ:(half + 1) * SQ] = res.results[core]["out"]
    return full


# revision 5
# speedup vs baseline: 1.0088x; 1.0088x over previous
"""MultiHeadAttention Trainium2 kernel.

Full inputs -> full output. Sharding: 8 cores = (batch b in 0..3) x (query
half in 0..1). Each core computes attention for its 1024 query rows of batch
b over all 2048 keys of batch b (K/V projections duplicated across the pair
of cores sharing a batch), then applies the output projection for its rows.
Outputs are disjoint row-slices of (B, S, D); host assembly is a pure concat.

Per-core math (all matmuls in float32r on the PE, fp32 PSUM accumulate):
  phase A: transpose x chunks via PE-identity, project Q^T [d,s], K^T [d,s],
           V [s,d] augmented with a ones column (K^T/V/Q^T round-trip DRAM),
           rank-1 matmuls add bq/bk.
  phase B: software-pipelined over iterations t=(head, 512-query-block):
           logits^T [sk,sq] = K_h^T.T @ Q_h^T and P=exp(...) for iteration t
           interleave with ctx matmuls of t-1 (so the PE never waits on the
           scalar engine) and the normalization of t-2 (so the PE never
           waits on the reciprocal).
  phase C: out = ctx @ wo + bo (ctx^T chunks are the stationary operand).
"""

import numpy as np

import concourse.bass as bass
import concourse.mybir as mybir
import concourse.tile as tile
from concourse import bacc
from concourse.bass_utils import run_bass_kernel_spmd

f32 = mybir.dt.float32
f32r = mybir.dt.float32r

B, S, D, H, DH = 4, 2048, 1024, 16, 64
SQ = S // 2          # query rows per core
N_CORES = 8
Exp = mybir.ActivationFunctionType.Exp

KC = D // 128        # 8 contraction chunks
SKC = S // 128       # 16 key chunks
NT = 2 * H           # 32 pipelined iterations (head, sqb)


def _build():
    nc = bacc.Bacc(None, target_bir_lowering=False)

    xq = nc.dram_tensor("xq", [SQ, D], f32r, kind="ExternalInput")
    xk = nc.dram_tensor("xk", [S, D], f32r, kind="ExternalInput")
    xv = nc.dram_tensor("xv", [S, D], f32r, kind="ExternalInput")
    wq = nc.dram_tensor("wq", [D, D], f32r, kind="ExternalInput")
    wk = nc.dram_tensor("wk", [D, D], f32r, kind="ExternalInput")
    wv = nc.dram_tensor("wv", [D, D], f32r, kind="ExternalInput")
    wo = nc.dram_tensor("wo", [D, D], f32r, kind="ExternalInput")
    bq = nc.dram_tensor("bq", [1, D], f32r, kind="ExternalInput")
    bk = nc.dram_tensor("bk", [1, D], f32r, kind="ExternalInput")
    bo = nc.dram_tensor("bo", [1, D], f32r, kind="ExternalInput")
    bvT = nc.dram_tensor("bvT", [DH, H], f32, kind="ExternalInput")   # bv[h*64+p] at [p, h]
    mb = nc.dram_tensor("mb", [128, SKC], f32, kind="ExternalInput")  # mask*-1e9 at [p, chunk]
    idn = nc.dram_tensor("idn", [128, 128], f32r, kind="ExternalInput")
    one = nc.dram_tensor("one", [128, 512], f32r, kind="ExternalInput")
    out = nc.dram_tensor("out", [SQ, D], f32, kind="ExternalOutput")

    with tile.TileContext(nc) as tc:
        _emit(nc, tc, xq, xk, xv, wq, wk, wv, wo, bq, bk, bo, bvT, mb, idn, one, out)
    nc.finalize()
    return nc


def _emit(nc, tc, xq, xk, xv, wq, wk, wv, wo, bq, bk, bo, bvT, mb, idn, one, out):
    from contextlib import ExitStack

    with ExitStack() as ctx:
        consts = ctx.enter_context(tc.tile_pool(name="consts", bufs=1))
        wpool = ctx.enter_context(tc.tile_pool(name="wpool", bufs=1))
        xload = ctx.enter_context(tc.tile_pool(name="xload", bufs=2))
        xtp = ctx.enter_context(tc.tile_pool(name="xtp", bufs=1))
        qts = ctx.enter_context(tc.tile_pool(name="qts", bufs=2))
        kts = ctx.enter_context(tc.tile_pool(name="kts", bufs=2))
        vas = ctx.enter_context(tc.tile_pool(name="vas", bufs=2))
        ptp = ctx.enter_context(tc.tile_pool(name="ptp", bufs=24))
        stg = ctx.enter_context(tc.tile_pool(name="stg", bufs=3))
        nrm = ctx.enter_context(tc.tile_pool(name="nrm", bufs=3))
        cns = ctx.enter_context(tc.tile_pool(name="cns", bufs=3))
        psA = ctx.enter_context(tc.tile_pool(name="psA", bufs=3, space="PSUM"))
        psT = ctx.enter_context(tc.tile_pool(name="psT", bufs=2, space="PSUM"))
        psC = ctx.enter_context(tc.tile_pool(name="psC", bufs=2, space="PSUM"))
        psB = ctx.enter_context(tc.tile_pool(name="psB", bufs=1, space="PSUM"))
        dram = ctx.enter_context(tc.tile_pool(name="dram", bufs=1, space="DRAM"))

        ktd = dram.tile([D, S], f32r)                 # K^T
        vad = dram.tile([SKC, 128, H, DH + 1], f32r)  # V augmented with ones col
        qtd = dram.tile([D, SQ], f32r)                # Q^T
        cnd = dram.tile([D, SQ], f32r)                # normalized ctx^T

        ident = consts.tile([128, 128], f32r)
        nc.sync.dma_start(ident, idn[:])
        ones = consts.tile([128, 512], f32r)
        nc.sync.dma_start(ones, one[:])
        bq_sb = consts.tile([1, D], f32r)
        nc.sync.dma_start(bq_sb, bq[:])
        bk_sb = consts.tile([1, D], f32r)
        nc.sync.dma_start(bk_sb, bk[:])
        bo_sb = consts.tile([1, D], f32r)
        nc.sync.dma_start(bo_sb, bo[:])
        bv_sb = consts.tile([DH, H], f32)
        nc.sync.dma_start(bv_sb, bvT[:])
        mb_sb = consts.tile([128, SKC], f32)
        nc.sync.dma_start(mb_sb, mb[:])

        # ---- helper: transpose a [128, 512] slice of a row-block into xT
        def transpose_block(xrow, kc0, xT, col0):
            pst = psT.tile([128, 512], f32r, tag="psT", name="pst")
            for j in range(4):
                nc.tensor.matmul(
                    pst[:, j * 128:(j + 1) * 128],
                    lhsT=xrow[:, (kc0 + j) * 128:(kc0 + j + 1) * 128],
                    rhs=ident[:], is_transpose=True, start=True, stop=True,
                )
            nc.vector.tensor_copy(
                xT[:, kc0:kc0 + 4, col0:col0 + 128],
                pst.rearrange("p (j c) -> p j c", j=4),
            )

        # ================= phase A =================
        def project_T(w_sb, b_sb, x_dram, blk, dst_dram):
            """One 512-row block of a transposed projection -> dst_dram."""
            xT = xtp.tile([128, KC, 512], f32r, tag="xT", name="xT")
            for st in range(4):
                xrow = xload.tile([128, D], f32r, tag="xr", name="xrow")
                nc.sync.dma_start(xrow, x_dram[(blk * 4 + st) * 128:(blk * 4 + st + 1) * 128, :])
                transpose_block(xrow, 0, xT, st * 128)
                transpose_block(xrow, 4, xT, st * 128)
            for dc in range(KC):
                ps = psA.tile([128, 512], f32, tag="psA", name="ps")
                for kc in range(KC):
                    nc.tensor.matmul(ps, lhsT=w_sb[:, kc, dc * 128:(dc + 1) * 128],
                                     rhs=xT[:, kc, :], start=(kc == 0), stop=False)
                nc.tensor.matmul(ps, lhsT=b_sb[0:1, dc * 128:(dc + 1) * 128],
                                 rhs=ones[0:1, :], start=False, stop=True)
                st_t = stg.tile([128, 512], f32r, tag="stg", name="st_t")
                nc.vector.tensor_copy(st_t, ps)
                nc.sync.dma_start(
                    dst_dram[dc * 128:(dc + 1) * 128, blk * 512:(blk + 1) * 512], st_t)

        wq_sb = wpool.tile([128, KC, D], f32r, tag="w", name="wq_sb")
        nc.sync.dma_start(wq_sb, wq.rearrange("(ko p) d -> p ko d", p=128))
        for sqb in range(2):
            project_T(wq_sb, bq_sb, xq, sqb, qtd)

        wk_sb = wpool.tile([128, KC, D], f32r, tag="w", name="wk_sb")
        nc.sync.dma_start(wk_sb, wk.rearrange("(ko p) d -> p ko d", p=128))
        for skb in range(4):
            project_T(wk_sb, bk_sb, xk, skb, ktd)

        wv_sb = wpool.tile([128, KC, D], f32r, tag="w", name="wv_sb")
        nc.sync.dma_start(wv_sb, wv.rearrange("(ko p) d -> p ko d", p=128))
        for sc in range(SKC):
            nc.sync.dma_start(vad[sc, :, :, DH], ones[:, 0:H])
        for sc in range(SKC):
            xT = xtp.tile([128, KC, 512], f32r, tag="xT", name="xT")
            xrow = xload.tile([128, D], f32r, tag="xr", name="xrow")
            nc.sync.dma_start(xrow, xv[sc * 128:(sc + 1) * 128, :])
            transpose_block(xrow, 0, xT, 0)
            transpose_block(xrow, 4, xT, 0)
            for dh2 in range(2):
                ps = psA.tile([128, 512], f32, tag="psA", name="ps")
                for kc in range(KC):
                    nc.tensor.matmul(ps, lhsT=xT[:, kc, 0:128],
                                     rhs=wv_sb[:, kc, dh2 * 512:(dh2 + 1) * 512],
                                     start=(kc == 0), stop=(kc == KC - 1))
                st_t = stg.tile([128, 512], f32r, tag="stg", name="st_t")
                nc.vector.tensor_copy(st_t, ps)
                nc.sync.dma_start(
                    vad[sc, :, dh2 * 8:(dh2 + 1) * 8, 0:DH],
                    st_t.rearrange("p (h d) -> p h d", h=8),
                )

        # ================= phase B: software-pipelined attention =========
        # iteration t: h = t//2, sqb = t%2.  During block t the PE runs
        # logits(t) interleaved with ctx(t-1); the norm of t-2 is emitted
        # mid-block (its reciprocal, issued at the end of block t-1's ctx,
        # has had time to finish on the DVE).
        state = {}   # t -> dict(pt=list of 16 tiles, psc=..., base=..., h=..., sqb=...)

        def emit_logits_chunk(t, skc):
            st_ = state[t]
            psl = psA.tile([128, 512], f32, tag="psA", name="psl")
            nc.tensor.matmul(psl,
                             lhsT=st_["kt"][st_["base"]:st_["base"] + 64,
                                            skc * 128:(skc + 1) * 128],
                             rhs=st_["qt"][st_["base"]:st_["base"] + 64, :],
                             start=True, stop=True)
            pt_t = ptp.tile([128, 512], f32r, tag="pt", name="pt_t")
            nc.scalar.activation(pt_t, psl, Exp, bias=mb_sb[:, skc:skc + 1], scale=0.125)
            st_["pt"].append(pt_t)

        def emit_ctx_chunk(t, skc):
            st_ = state[t]
            if skc == 0:
                st_["psc"] = psC.tile([128, 512], f32, tag="psC", name="psc")
            nc.tensor.matmul(st_["psc"][0:DH + 1, :], lhsT=st_["va"][:, skc, :],
                             rhs=st_["pt"][skc][:],
                             start=(skc == 0), stop=(skc == SKC - 1))

        def emit_recip(t):
            st_ = state[t]
            recf = nrm.tile([65, 512], f32, tag="recf", name="recf")
            nc.vector.reciprocal(recf[64:65, :], st_["psc"][64:65, :])
            rec = nrm.tile([65, 512], f32r, tag="rec", name="rec")
            with nc.allow_low_precision(reason="softmax recip rounded to f32r"):
                nc.vector.tensor_copy(rec[64:65, :], recf[64:65, :])
            st_["rec"] = rec

        def emit_norm(t):
            st_ = state[t]
            h, sqb = st_["h"], st_["sqb"]
            psb = psB.tile([128, 512], f32, tag="psB", name="psb")
            nc.tensor.matmul(psb[0:64, :], lhsT=ones[64:65, 0:64],
                             rhs=st_["rec"][64:65, :], start=True, stop=True)
            bc = nrm.tile([64, 512], f32, tag="bc", name="bc")
            nc.vector.tensor_copy(bc, psb[0:64, :])
            cn = cns.tile([64, 512], f32r, tag="cn", name="cn")
            with nc.allow_low_precision(reason="ctx rounded to f32r for out proj"):
                nc.vector.tensor_mul(out=cn, in0=st_["psc"][0:DH, :], in1=bc)
                nc.vector.tensor_scalar_add(cn, cn, bv_sb[0:DH, h:h + 1])
            nc.sync.dma_start(cnd[h * 64:(h + 1) * 64, sqb * 512:(sqb + 1) * 512], cn)
            del state[t]

        cur_kt = cur_va = None
        for t in range(NT):
            h, sqb = divmod(t, 2)
            base = (h % 2) * 64
            st_ = state[t] = {"h": h, "sqb": sqb, "base": base, "pt": []}
            if sqb == 0:
                cur_kt = kts.tile([128, S], f32r, tag="kt", name="kt")
                nc.sync.dma_start(cur_kt[base:base + 64, :], ktd[h * 64:(h + 1) * 64, :])
                cur_va = vas.tile([128, SKC, DH + 1], f32r, tag="va", name="va")
                nc.sync.dma_start(cur_va, vad[:, :, h, :].rearrange("sc p c -> p sc c"))
            st_["kt"], st_["va"] = cur_kt, cur_va
            qt = qts.tile([128, 512], f32r, tag="qt", name="qt")
            nc.sync.dma_start(qt[base:base + 64, :],
                              qtd[h * 64:(h + 1) * 64, sqb * 512:(sqb + 1) * 512])
            st_["qt"] = qt

            for skc in range(SKC):
                emit_logits_chunk(t, skc)
                if t >= 1:
                    emit_ctx_chunk(t - 1, skc)
                if t >= 2 and skc == 8:
                    emit_norm(t - 2)
            if t >= 1 and t - 1 in state:
                emit_recip(t - 1)

        # pipeline drain
        for skc in range(SKC):
            emit_ctx_chunk(NT - 1, skc)
        emit_norm(NT - 2)
        emit_recip(NT - 1)
        emit_norm(NT - 1)

        # ================= phase C: output projection =================
        wo_sb = wpool.tile([128, KC, D], f32r, tag="w", name="wo_sb")
        nc.sync.dma_start(wo_sb, wo.rearrange("(ko p) d -> p ko d", p=128))
        for st8 in range(8):
            cT = cns.tile([128, KC, 128], f32r, tag="cT", name="cT")
            nc.sync.dma_start(cT, cnd[:, st8 * 128:(st8 + 1) * 128]
                              .rearrange("(ko p) s -> p ko s", p=128))
            for dh2 in range(2):
                ps = psA.tile([128, 512], f32, tag="psA", name="ps")
                for kc in range(KC):
                    nc.tensor.matmul(ps, lhsT=cT[:, kc, :],
                                     rhs=wo_sb[:, kc, dh2 * 512:(dh2 + 1) * 512],
                                     start=(kc == 0), stop=False)
                nc.tensor.matmul(ps, lhsT=ones[0:1, 0:128],
                                 rhs=bo_sb[0:1, dh2 * 512:(dh2 + 1) * 512],
                                 start=False, stop=True)
                st_t = stg.tile([128, 512], f32, tag="ost", name="ost")
                nc.vector.tensor_copy(st_t, ps)
                nc.sync.dma_start(out[st8 * 128:(st8 + 1) * 128,
                                      dh2 * 512:(dh2 + 1) * 512], st_t)


_NC_CACHE = None


def kernel(query, key, value, mask, wq, bq, wk, bk, wv, bv, wo, bo):
    global _NC_CACHE
    if _NC_CACHE is None:
        _NC_CACHE = _build()
    nc = _NC_CACHE

    query = np.ascontiguousarray(np.asarray(query, dtype=np.float32))
    key = np.ascontiguousarray(np.asarray(key, dtype=np.float32))
    value = np.ascontiguousarray(np.asarray(value, dtype=np.float32))
    mask = np.asarray(mask, dtype=np.float32)

    shared = {
        "wq": np.asarray(wq, np.float32), "wk": np.asarray(wk, np.float32),
        "wv": np.asarray(wv, np.float32), "wo": np.asarray(wo, np.float32),
        "bq": np.asarray(bq, np.float32).reshape(1, D),
        "bk": np.asarray(bk, np.float32).reshape(1, D),
        "bo": np.asarray(bo, np.float32).reshape(1, D),
        "bvT": np.ascontiguousarray(np.asarray(bv, np.float32).reshape(H, DH).T),
        "idn": np.eye(128, dtype=np.float32),
        "one": np.ones((128, 512), np.float32),
    }
    in_maps = []
    for core in range(N_CORES):
        b, half = divmod(core, 2)
        mbc = np.ascontiguousarray(
            (mask[b, 0, 0] * np.float32(-1e9)).reshape(S // 128, 128).T)
        in_maps.append({
            "xq": np.ascontiguousarray(query[b, half * SQ:(half + 1) * SQ]),
            "xk": key[b], "xv": value[b], "mb": mbc, **shared,
        })

    res = run_bass_kernel_spmd(nc, in_maps, core_ids=list(range(N_CORES)))
    full = np.empty((B, S, D), np.float32)
    for core in range(N_CORES):
        b, half = divmod(core, 2)
        full[b, half * SQ:(half + 1) * SQ] = res.results[core]["out"]
    return full


# revision 7
# speedup vs baseline: 1.1861x; 1.1758x over previous
"""MultiHeadAttention Trainium2 kernel.

Full inputs -> full output. Sharding: 8 cores = (batch b in 0..3) x (query
half in 0..1). Each core computes attention for its 1024 query rows of batch
b over all 2048 keys of batch b (K/V projections duplicated across the pair
of cores sharing a batch), then applies the output projection for its rows.
Outputs are disjoint row-slices of (B, S, D); host assembly is a pure concat.

Per-core math (all matmuls in float32r on the PE, fp32 PSUM accumulate):
  phase A: transpose x chunks via PE-identity, project Q^T [d,s], K^T [d,s],
           V [s,d] augmented with a ones column (K^T/V/Q^T round-trip DRAM),
           rank-1 matmuls add bq/bk.
  phase B: software-pipelined over iterations t=(head, 512-query-block):
           logits^T [sk,sq] = K_h^T.T @ Q_h^T and P=exp(...) for iteration t
           interleave with ctx matmuls of t-1 (so the PE never waits on the
           scalar engine) and the normalization of t-2 (so the PE never
           waits on the reciprocal).
  phase C: out = ctx @ wo + bo (ctx^T chunks are the stationary operand).
"""

import numpy as np

import concourse.bass as bass
import concourse.mybir as mybir
import concourse.tile as tile
from concourse import bacc
from concourse.bass_utils import run_bass_kernel_spmd

f32 = mybir.dt.float32
f32r = mybir.dt.float32r

B, S, D, H, DH = 4, 2048, 1024, 16, 64
SQ = S // 2          # query rows per core
N_CORES = 8
Exp = mybir.ActivationFunctionType.Exp

KC = D // 128        # 8 contraction chunks
SKC = S // 128       # 16 key chunks
NT = 2 * H           # 32 pipelined iterations (head, sqb)


def _build():
    nc = bacc.Bacc(None, target_bir_lowering=False)

    xq = nc.dram_tensor("xq", [SQ, D], f32r, kind="ExternalInput")
    xk = nc.dram_tensor("xk", [S, D], f32r, kind="ExternalInput")
    xv = nc.dram_tensor("xv", [S, D], f32r, kind="ExternalInput")
    wq = nc.dram_tensor("wq", [D, D], f32r, kind="ExternalInput")
    wk = nc.dram_tensor("wk", [D, D], f32r, kind="ExternalInput")
    wv = nc.dram_tensor("wv", [D, D], f32r, kind="ExternalInput")
    wo = nc.dram_tensor("wo", [D, D], f32r, kind="ExternalInput")
    bq = nc.dram_tensor("bq", [1, D], f32r, kind="ExternalInput")
    bk = nc.dram_tensor("bk", [1, D], f32r, kind="ExternalInput")
    bo = nc.dram_tensor("bo", [1, D], f32r, kind="ExternalInput")
    bvT = nc.dram_tensor("bvT", [DH, H], f32, kind="ExternalInput")   # bv[h*64+p] at [p, h]
    mb = nc.dram_tensor("mb", [128, SKC], f32, kind="ExternalInput")  # mask*-1e9 at [p, chunk]
    idn = nc.dram_tensor("idn", [128, 128], f32r, kind="ExternalInput")
    one = nc.dram_tensor("one", [128, 512], f32r, kind="ExternalInput")
    out = nc.dram_tensor("out", [SQ, D], f32, kind="ExternalOutput")

    with tile.TileContext(nc) as tc:
        _emit(nc, tc, xq, xk, xv, wq, wk, wv, wo, bq, bk, bo, bvT, mb, idn, one, out)
    nc.finalize()
    return nc


def _emit(nc, tc, xq, xk, xv, wq, wk, wv, wo, bq, bk, bo, bvT, mb, idn, one, out):
    from contextlib import ExitStack

    with ExitStack() as ctx:
        consts = ctx.enter_context(tc.tile_pool(name="consts", bufs=1))
        wpool = ctx.enter_context(tc.tile_pool(name="wpool", bufs=1))
        xload = ctx.enter_context(tc.tile_pool(name="xload", bufs=2))
        xtp = ctx.enter_context(tc.tile_pool(name="xtp", bufs=1))
        qts = ctx.enter_context(tc.tile_pool(name="qts", bufs=2))
        kts = ctx.enter_context(tc.tile_pool(name="kts", bufs=2))
        vas = ctx.enter_context(tc.tile_pool(name="vas", bufs=2))
        ptp = ctx.enter_context(tc.tile_pool(name="ptp", bufs=14))
        stg = ctx.enter_context(tc.tile_pool(name="stg", bufs=3))
        nrm = ctx.enter_context(tc.tile_pool(name="nrm", bufs=3))
        cns = ctx.enter_context(tc.tile_pool(name="cns", bufs=3))
        cns2 = ctx.enter_context(tc.tile_pool(name="cns2", bufs=2))
        psA = ctx.enter_context(tc.tile_pool(name="psA", bufs=2, space="PSUM"))
        psT = ctx.enter_context(tc.tile_pool(name="psT", bufs=1, space="PSUM"))
        psC = ctx.enter_context(tc.tile_pool(name="psC", bufs=2, space="PSUM"))
        psB = ctx.enter_context(tc.tile_pool(name="psB", bufs=1, space="PSUM"))
        dram = ctx.enter_context(tc.tile_pool(name="dram", bufs=1, space="DRAM"))

        ktd = dram.tile([D, S], f32r)                 # K^T
        vad = dram.tile([SKC, 128, H, DH + 1], f32r)  # V augmented with ones col
        qtd = dram.tile([D, SQ], f32r)                # Q^T
        cnd = dram.tile([D, SQ], f32r)                # normalized ctx^T

        ident = consts.tile([128, 128], f32r)
        nc.sync.dma_start(ident, idn[:])
        ones = consts.tile([128, 512], f32r)
        nc.sync.dma_start(ones, one[:])
        bq_sb = consts.tile([1, D], f32r)
        nc.sync.dma_start(bq_sb, bq[:])
        bk_sb = consts.tile([1, D], f32r)
        nc.sync.dma_start(bk_sb, bk[:])
        bo_sb = consts.tile([1, D], f32r)
        nc.sync.dma_start(bo_sb, bo[:])
        bv_sb = consts.tile([DH, H], f32)
        nc.sync.dma_start(bv_sb, bvT[:])
        mb_sb = consts.tile([128, SKC], f32)
        nc.sync.dma_start(mb_sb, mb[:])

        # ---- helper: transpose a [128, 512] slice of a row-block into xT
        def transpose_block(xrow, kc0, xT, col0):
            pst = psT.tile([128, 512], f32r, tag="psT", name="pst")
            for j in range(4):
                nc.tensor.matmul(
                    pst[:, j * 128:(j + 1) * 128],
                    lhsT=xrow[:, (kc0 + j) * 128:(kc0 + j + 1) * 128],
                    rhs=ident[:], is_transpose=True, start=True, stop=True,
                )
            nc.vector.tensor_copy(
                xT[:, kc0:kc0 + 4, col0:col0 + 128],
                pst.rearrange("p (j c) -> p j c", j=4),
            )

        # ================= phase A =================
        def project_T(w_sb, b_sb, x_dram, blk, dst_dram):
            """One 512-row block of a transposed projection -> dst_dram."""
            xT = xtp.tile([128, KC, 512], f32r, tag="xT", name="xT")
            for st in range(4):
                xrow = xload.tile([128, D], f32r, tag="xr", name="xrow")
                nc.sync.dma_start(xrow, x_dram[(blk * 4 + st) * 128:(blk * 4 + st + 1) * 128, :])
                transpose_block(xrow, 0, xT, st * 128)
                transpose_block(xrow, 4, xT, st * 128)
            for dc2 in range(KC // 2):
                ps = psA.tile([128, 1024], f32, tag="psA", name="ps")
                for half in range(2):
                    dc = dc2 * 2 + half
                    ph = ps[:, half * 512:(half + 1) * 512]
                    for kc in range(KC):
                        nc.tensor.matmul(ph, lhsT=w_sb[:, kc, dc * 128:(dc + 1) * 128],
                                         rhs=xT[:, kc, :], start=(kc == 0), stop=False)
                    nc.tensor.matmul(ph, lhsT=b_sb[0:1, dc * 128:(dc + 1) * 128],
                                     rhs=ones[0:1, :], start=False, stop=True)
                st_t = stg.tile([128, 1024], f32r, tag="stg", name="st_t")
                nc.vector.tensor_copy(st_t, ps)
                for half in range(2):
                    dc = dc2 * 2 + half
                    nc.sync.dma_start(
                        dst_dram[dc * 128:(dc + 1) * 128, blk * 512:(blk + 1) * 512],
                        st_t[:, half * 512:(half + 1) * 512])

        wq_sb = wpool.tile([128, KC, D], f32r, tag="w", name="wq_sb")
        nc.sync.dma_start(wq_sb, wq.rearrange("(ko p) d -> p ko d", p=128))
        for sqb in range(2):
            project_T(wq_sb, bq_sb, xq, sqb, qtd)

        wk_sb = wpool.tile([128, KC, D], f32r, tag="w", name="wk_sb")
        nc.sync.dma_start(wk_sb, wk.rearrange("(ko p) d -> p ko d", p=128))
        for skb in range(4):
            project_T(wk_sb, bk_sb, xk, skb, ktd)

        wv_sb = wpool.tile([128, KC, D], f32r, tag="w", name="wv_sb")
        nc.sync.dma_start(wv_sb, wv.rearrange("(ko p) d -> p ko d", p=128))
        for sc in range(SKC):
            nc.sync.dma_start(vad[sc, :, :, DH], ones[:, 0:H])
        for sc in range(SKC):
            xT = xtp.tile([128, KC, 512], f32r, tag="xT", name="xT")
            xrow = xload.tile([128, D], f32r, tag="xr", name="xrow")
            nc.sync.dma_start(xrow, xv[sc * 128:(sc + 1) * 128, :])
            transpose_block(xrow, 0, xT, 0)
            transpose_block(xrow, 4, xT, 0)
            ps = psA.tile([128, 1024], f32, tag="psA", name="ps")
            for dh2 in range(2):
                ph = ps[:, dh2 * 512:(dh2 + 1) * 512]
                for kc in range(KC):
                    nc.tensor.matmul(ph, lhsT=xT[:, kc, 0:128],
                                     rhs=wv_sb[:, kc, dh2 * 512:(dh2 + 1) * 512],
                                     start=(kc == 0), stop=(kc == KC - 1))
            st_t = stg.tile([128, 1024], f32r, tag="stg", name="st_t")
            nc.vector.tensor_copy(st_t, ps)
            nc.sync.dma_start(
                vad[sc, :, :, 0:DH],
                st_t.rearrange("p (h d) -> p h d", h=16),
            )

        # ================= phase B: software-pipelined attention =========
        # iteration t: h = t//2, sqb = t%2.  During block t the PE runs
        # logits(t) interleaved with ctx(t-1); the norm of t-2 is emitted
        # mid-block (its reciprocal, issued at the end of block t-1's ctx,
        # has had time to finish on the DVE).
        state = {}   # t -> dict(pt=list of 16 tiles, psc=..., base=..., h=..., sqb=...)

        def emit_logits_pair(t, skc2):
            st_ = state[t]
            psl = psA.tile([128, 1024], f32, tag="psA", name="psl")
            for half in range(2):
                skc = skc2 * 2 + half
                nc.tensor.matmul(psl[:, half * 512:(half + 1) * 512],
                                 lhsT=st_["kt"][st_["base"]:st_["base"] + 64,
                                                skc * 128:(skc + 1) * 128],
                                 rhs=st_["qt"][st_["base"]:st_["base"] + 64, :],
                                 start=True, stop=True)
            pt_t = ptp.tile([128, 2, 512], f32r, tag="pt", name="pt_t")
            nc.scalar.activation(
                pt_t.rearrange("p a b -> p (a b)"), psl, Exp,
                bias=mb_sb[:, skc2 * 2:skc2 * 2 + 1], scale=0.125)
            st_["pt"].append(pt_t)

        def emit_ctx_chunk(t, skc):
            st_ = state[t]
            if skc == 0:
                st_["psc"] = psC.tile([128, 512], f32, tag="psC", name="psc")
            nc.tensor.matmul(st_["psc"][0:DH + 1, :], lhsT=st_["va"][:, skc, :],
                             rhs=st_["pt"][skc // 2][:, skc % 2, :],
                             start=(skc == 0), stop=(skc == SKC - 1))

        def emit_recip(t):
            st_ = state[t]
            recf = nrm.tile([65, 512], f32, tag="recf", name="recf")
            nc.vector.reciprocal(recf[64:65, :], st_["psc"][64:65, :])
            rec = nrm.tile([65, 512], f32r, tag="rec", name="rec")
            with nc.allow_low_precision(reason="softmax recip rounded to f32r"):
                nc.vector.tensor_copy(rec[64:65, :], recf[64:65, :])
            st_["rec"] = rec

        def emit_norm(t):
            st_ = state[t]
            h, sqb = st_["h"], st_["sqb"]
            psb = psB.tile([128, 512], f32, tag="psB", name="psb")
            nc.tensor.matmul(psb[0:64, :], lhsT=ones[64:65, 0:64],
                             rhs=st_["rec"][64:65, :], start=True, stop=True)
            bc = nrm.tile([64, 512], f32, tag="bc", name="bc")
            nc.vector.tensor_copy(bc, psb[0:64, :])
            cn = cns.tile([64, 512], f32r, tag="cn", name="cn")
            with nc.allow_low_precision(reason="ctx rounded to f32r for out proj"):
                nc.vector.tensor_mul(out=cn, in0=st_["psc"][0:DH, :], in1=bc)
                nc.vector.tensor_scalar_add(cn, cn, bv_sb[0:DH, h:h + 1])
            nc.sync.dma_start(cnd[h * 64:(h + 1) * 64, sqb * 512:(sqb + 1) * 512], cn)
            del state[t]

        cur_kt = cur_va = None
        for t in range(NT):
            h, sqb = divmod(t, 2)
            base = (h % 2) * 64
            st_ = state[t] = {"h": h, "sqb": sqb, "base": base, "pt": []}
            if sqb == 0:
                cur_kt = kts.tile([128, S], f32r, tag="kt", name="kt")
                nc.sync.dma_start(cur_kt[base:base + 64, :], ktd[h * 64:(h + 1) * 64, :])
                cur_va = vas.tile([128, SKC, DH + 1], f32r, tag="va", name="va")
                nc.sync.dma_start(cur_va, vad[:, :, h, :].rearrange("sc p c -> p sc c"))
            st_["kt"], st_["va"] = cur_kt, cur_va
            qt = qts.tile([128, 512], f32r, tag="qt", name="qt")
            nc.sync.dma_start(qt[base:base + 64, :],
                              qtd[h * 64:(h + 1) * 64, sqb * 512:(sqb + 1) * 512])
            st_["qt"] = qt

            for skc2 in range(SKC // 2):
                emit_logits_pair(t, skc2)
                if t >= 1:
                    emit_ctx_chunk(t - 1, skc2 * 2)
                    emit_ctx_chunk(t - 1, skc2 * 2 + 1)
                if t >= 2 and skc2 == 4:
                    emit_norm(t - 2)
            if t >= 1 and t - 1 in state:
                emit_recip(t - 1)

        # pipeline drain
        for skc in range(SKC):
            emit_ctx_chunk(NT - 1, skc)
        emit_norm(NT - 2)
        emit_recip(NT - 1)
        emit_norm(NT - 1)

        # ================= phase C: output projection =================
        wo_sb = wpool.tile([128, KC, D], f32r, tag="w", name="wo_sb")
        nc.sync.dma_start(wo_sb, wo.rearrange("(ko p) d -> p ko d", p=128))
        for st8 in range(8):
            cT = cns2.tile([128, KC, 128], f32r, tag="cT", name="cT")
            nc.sync.dma_start(cT, cnd[:, st8 * 128:(st8 + 1) * 128]
                              .rearrange("(ko p) s -> p ko s", p=128))
            for dh2 in range(2):
                ps = psA.tile([128, 512], f32, tag="psA", name="ps")
                for kc in range(KC):
                    nc.tensor.matmul(ps, lhsT=cT[:, kc, :],
                                     rhs=wo_sb[:, kc, dh2 * 512:(dh2 + 1) * 512],
                                     start=(kc == 0), stop=False)
                nc.tensor.matmul(ps, lhsT=ones[0:1, 0:128],
                                 rhs=bo_sb[0:1, dh2 * 512:(dh2 + 1) * 512],
                                 start=False, stop=True)
                st_t = stg.tile([128, 512], f32, tag="ost", name="ost")
                nc.vector.tensor_copy(st_t, ps)
                nc.sync.dma_start(out[st8 * 128:(st8 + 1) * 128,
                                      dh2 * 512:(dh2 + 1) * 512], st_t)


_NC_CACHE = None


def kernel(query, key, value, mask, wq, bq, wk, bk, wv, bv, wo, bo):
    global _NC_CACHE
    if _NC_CACHE is None:
        _NC_CACHE = _build()
    nc = _NC_CACHE

    query = np.ascontiguousarray(np.asarray(query, dtype=np.float32))
    key = np.ascontiguousarray(np.asarray(key, dtype=np.float32))
    value = np.ascontiguousarray(np.asarray(value, dtype=np.float32))
    mask = np.asarray(mask, dtype=np.float32)

    shared = {
        "wq": np.asarray(wq, np.float32), "wk": np.asarray(wk, np.float32),
        "wv": np.asarray(wv, np.float32), "wo": np.asarray(wo, np.float32),
        "bq": np.asarray(bq, np.float32).reshape(1, D),
        "bk": np.asarray(bk, np.float32).reshape(1, D),
        "bo": np.asarray(bo, np.float32).reshape(1, D),
        "bvT": np.ascontiguousarray(np.asarray(bv, np.float32).reshape(H, DH).T),
        "idn": np.eye(128, dtype=np.float32),
        "one": np.ones((128, 512), np.float32),
    }
    in_maps = []
    for core in range(N_CORES):
        b, half = divmod(core, 2)
        mbc = np.ascontiguousarray(
            (mask[b, 0, 0] * np.float32(-1e9)).reshape(S // 128, 128).T)
        in_maps.append({
            "xq": np.ascontiguousarray(query[b, half * SQ:(half + 1) * SQ]),
            "xk": key[b], "xv": value[b], "mb": mbc, **shared,
        })

    res = run_bass_kernel_spmd(nc, in_maps, core_ids=list(range(N_CORES)))
    full = np.empty((B, S, D), np.float32)
    for core in range(N_CORES):
        b, half = divmod(core, 2)
        full[b, half * SQ:(half + 1) * SQ] = res.results[core]["out"]
    return full


# revision 9
# speedup vs baseline: 1.1933x; 1.0060x over previous
"""MultiHeadAttention Trainium2 kernel.

Full inputs -> full output. Sharding: 8 cores = (batch b in 0..3) x (query
half in 0..1). Each core computes attention for its 1024 query rows of batch
b over all 2048 keys of batch b (K/V projections duplicated across the pair
of cores sharing a batch), then applies the output projection for its rows.
Outputs are disjoint row-slices of (B, S, D); host assembly is a pure concat.

Per-core math (all matmuls in float32r on the PE, fp32 PSUM accumulate):
  phase A: transpose x chunks via PE-identity, project Q^T [d,s], K^T [d,s],
           V [s,d] augmented with a ones column (K^T/V/Q^T round-trip DRAM),
           rank-1 matmuls add bq/bk.
  phase B: software-pipelined over iterations t=(head, 512-query-block):
           logits^T [sk,sq] = K_h^T.T @ Q_h^T and P=exp(...) for iteration t
           interleave with ctx matmuls of t-1 (so the PE never waits on the
           scalar engine) and the normalization of t-2 (so the PE never
           waits on the reciprocal).
  phase C: out = ctx @ wo + bo (ctx^T chunks are the stationary operand).
"""

import numpy as np

import concourse.bass as bass
import concourse.mybir as mybir
import concourse.tile as tile
from concourse import bacc
from concourse.bass_utils import run_bass_kernel_spmd

f32 = mybir.dt.float32
f32r = mybir.dt.float32r

B, S, D, H, DH = 4, 2048, 1024, 16, 64
SQ = S // 2          # query rows per core
N_CORES = 8
Exp = mybir.ActivationFunctionType.Exp

KC = D // 128        # 8 contraction chunks
SKC = S // 128       # 16 key chunks
NT = 2 * H           # 32 pipelined iterations (head, sqb)


def _build():
    nc = bacc.Bacc(None, target_bir_lowering=False)

    xq = nc.dram_tensor("xq", [SQ, D], f32r, kind="ExternalInput")
    xk = nc.dram_tensor("xk", [S, D], f32r, kind="ExternalInput")
    xv = nc.dram_tensor("xv", [S, D], f32r, kind="ExternalInput")
    wq = nc.dram_tensor("wq", [D, D], f32r, kind="ExternalInput")
    wk = nc.dram_tensor("wk", [D, D], f32r, kind="ExternalInput")
    wv = nc.dram_tensor("wv", [D, D], f32r, kind="ExternalInput")
    wo = nc.dram_tensor("wo", [D, D], f32r, kind="ExternalInput")
    bq = nc.dram_tensor("bq", [1, D], f32r, kind="ExternalInput")
    bk = nc.dram_tensor("bk", [1, D], f32r, kind="ExternalInput")
    bo = nc.dram_tensor("bo", [1, D], f32r, kind="ExternalInput")
    bvT = nc.dram_tensor("bvT", [DH, H], f32, kind="ExternalInput")   # bv[h*64+p] at [p, h]
    mb = nc.dram_tensor("mb", [128, SKC], f32, kind="ExternalInput")  # mask*-1e9 at [p, chunk]
    idn = nc.dram_tensor("idn", [128, 128], f32r, kind="ExternalInput")
    one = nc.dram_tensor("one", [128, 512], f32r, kind="ExternalInput")
    out = nc.dram_tensor("out", [SQ, D], f32, kind="ExternalOutput")

    with tile.TileContext(nc) as tc:
        _emit(nc, tc, xq, xk, xv, wq, wk, wv, wo, bq, bk, bo, bvT, mb, idn, one, out)
    nc.finalize()
    return nc


def _emit(nc, tc, xq, xk, xv, wq, wk, wv, wo, bq, bk, bo, bvT, mb, idn, one, out):
    from contextlib import ExitStack

    with ExitStack() as ctx:
        consts = ctx.enter_context(tc.tile_pool(name="consts", bufs=1))
        wpool = ctx.enter_context(tc.tile_pool(name="wpool", bufs=1))
        xload = ctx.enter_context(tc.tile_pool(name="xload", bufs=2))
        xtp = ctx.enter_context(tc.tile_pool(name="xtp", bufs=1))
        qts = ctx.enter_context(tc.tile_pool(name="qts", bufs=2))
        kts = ctx.enter_context(tc.tile_pool(name="kts", bufs=2))
        vas = ctx.enter_context(tc.tile_pool(name="vas", bufs=2))
        ptp = ctx.enter_context(tc.tile_pool(name="ptp", bufs=13))
        stg = ctx.enter_context(tc.tile_pool(name="stg", bufs=3))
        nrm = ctx.enter_context(tc.tile_pool(name="nrm", bufs=3))
        cns = ctx.enter_context(tc.tile_pool(name="cns", bufs=3))
        cns2 = ctx.enter_context(tc.tile_pool(name="cns2", bufs=2))
        psA = ctx.enter_context(tc.tile_pool(name="psA", bufs=2, space="PSUM"))
        psX = ctx.enter_context(tc.tile_pool(name="psX", bufs=2, space="PSUM"))
        psC = ctx.enter_context(tc.tile_pool(name="psC", bufs=2, space="PSUM"))
        dram = ctx.enter_context(tc.tile_pool(name="dram", bufs=1, space="DRAM"))

        ktd = dram.tile([D, S], f32r)                 # K^T
        vad = dram.tile([SKC, 128, H, DH + 1], f32r)  # V augmented with ones col
        qtd = dram.tile([D, SQ], f32r)                # Q^T
        cnd = dram.tile([D, SQ], f32r)                # normalized ctx^T

        ident = consts.tile([128, 128], f32r)
        nc.sync.dma_start(ident, idn[:])
        ones = consts.tile([128, 512], f32r)
        nc.sync.dma_start(ones, one[:])
        bq_sb = consts.tile([1, D], f32r)
        nc.sync.dma_start(bq_sb, bq[:])
        bk_sb = consts.tile([1, D], f32r)
        nc.sync.dma_start(bk_sb, bk[:])
        bo_sb = consts.tile([1, D], f32r)
        nc.sync.dma_start(bo_sb, bo[:])
        bv_sb = consts.tile([DH, H], f32)
        nc.sync.dma_start(bv_sb, bvT[:])
        mb_sb = consts.tile([128, SKC], f32)
        nc.sync.dma_start(mb_sb, mb[:])

        # ---- helper: transpose a [128, 512] slice of a row-block into xT
        def transpose_block(xrow, kc0, xT, col0):
            pst = psX.tile([128, 512], f32r, tag="aux", name="pst")
            for j in range(4):
                nc.tensor.matmul(
                    pst[:, j * 128:(j + 1) * 128],
                    lhsT=xrow[:, (kc0 + j) * 128:(kc0 + j + 1) * 128],
                    rhs=ident[:], is_transpose=True, start=True, stop=True,
                )
            nc.vector.tensor_copy(
                xT[:, kc0:kc0 + 4, col0:col0 + 128],
                pst.rearrange("p (j c) -> p j c", j=4),
            )

        # ================= phase A =================
        def project_T(w_sb, b_sb, x_dram, blk, dst_dram):
            """One 512-row block of a transposed projection -> dst_dram."""
            xT = xtp.tile([128, KC, 512], f32r, tag="xT", name="xT")
            for st in range(4):
                xrow = xload.tile([128, D], f32r, tag="xr", name="xrow")
                nc.sync.dma_start(xrow, x_dram[(blk * 4 + st) * 128:(blk * 4 + st + 1) * 128, :])
                transpose_block(xrow, 0, xT, st * 128)
                transpose_block(xrow, 4, xT, st * 128)
            for dc2 in range(KC // 2):
                ps = psA.tile([128, 1024], f32, tag="psA", name="ps")
                for half in range(2):
                    dc = dc2 * 2 + half
                    ph = ps[:, half * 512:(half + 1) * 512]
                    for kc in range(KC):
                        nc.tensor.matmul(ph, lhsT=w_sb[:, kc, dc * 128:(dc + 1) * 128],
                                         rhs=xT[:, kc, :], start=(kc == 0), stop=False)
                    nc.tensor.matmul(ph, lhsT=b_sb[0:1, dc * 128:(dc + 1) * 128],
                                     rhs=ones[0:1, :], start=False, stop=True)
                st_t = stg.tile([128, 1024], f32r, tag="stg", name="st_t")
                nc.vector.tensor_copy(st_t, ps)
                for half in range(2):
                    dc = dc2 * 2 + half
                    nc.sync.dma_start(
                        dst_dram[dc * 128:(dc + 1) * 128, blk * 512:(blk + 1) * 512],
                        st_t[:, half * 512:(half + 1) * 512])

        wq_sb = wpool.tile([128, KC, D], f32r, tag="w", name="wq_sb")
        nc.sync.dma_start(wq_sb, wq.rearrange("(ko p) d -> p ko d", p=128))
        for sqb in range(2):
            project_T(wq_sb, bq_sb, xq, sqb, qtd)

        wk_sb = wpool.tile([128, KC, D], f32r, tag="w", name="wk_sb")
        nc.sync.dma_start(wk_sb, wk.rearrange("(ko p) d -> p ko d", p=128))
        for skb in range(4):
            project_T(wk_sb, bk_sb, xk, skb, ktd)

        wv_sb = wpool.tile([128, KC, D], f32r, tag="w", name="wv_sb")
        nc.sync.dma_start(wv_sb, wv.rearrange("(ko p) d -> p ko d", p=128))
        for sc in range(SKC):
            nc.sync.dma_start(vad[sc, :, :, DH], ones[:, 0:H])
        for sc in range(SKC):
            xT = xtp.tile([128, KC, 512], f32r, tag="xT", name="xT")
            xrow = xload.tile([128, D], f32r, tag="xr", name="xrow")
            nc.sync.dma_start(xrow, xv[sc * 128:(sc + 1) * 128, :])
            transpose_block(xrow, 0, xT, 0)
            transpose_block(xrow, 4, xT, 0)
            ps = psA.tile([128, 1024], f32, tag="psA", name="ps")
            for dh2 in range(2):
                ph = ps[:, dh2 * 512:(dh2 + 1) * 512]
                for kc in range(KC):
                    nc.tensor.matmul(ph, lhsT=xT[:, kc, 0:128],
                                     rhs=wv_sb[:, kc, dh2 * 512:(dh2 + 1) * 512],
                                     start=(kc == 0), stop=(kc == KC - 1))
            st_t = stg.tile([128, 1024], f32r, tag="stg", name="st_t")
            nc.vector.tensor_copy(st_t, ps)
            nc.sync.dma_start(
                vad[sc, :, :, 0:DH],
                st_t.rearrange("p (h d) -> p h d", h=16),
            )

        # ================= phase B: software-pipelined attention =========
        # iteration t: h = t//2, sqb = t%2.  During block t the PE runs
        # logits(t) interleaved with ctx(t-1); the norm of t-2 is emitted
        # mid-block (its reciprocal, issued at the end of block t-1's ctx,
        # has had time to finish on the DVE).
        state = {}   # t -> dict(pt=list of 16 tiles, psc=..., base=..., h=..., sqb=...)

        def emit_logits_pair(t, skc2):
            st_ = state[t]
            psl = psA.tile([128, 1024], f32, tag="psA", name="psl")
            for half in range(2):
                skc = skc2 * 2 + half
                nc.tensor.matmul(psl[:, half * 512:(half + 1) * 512],
                                 lhsT=st_["kt"][st_["base"]:st_["base"] + 64,
                                                skc * 128:(skc + 1) * 128],
                                 rhs=st_["qt"][st_["base"]:st_["base"] + 64, :],
                                 start=True, stop=True)
            pt_t = ptp.tile([128, 2, 512], f32r, tag="pt", name="pt_t")
            nc.scalar.activation(
                pt_t.rearrange("p a b -> p (a b)"), psl, Exp,
                bias=mb_sb[:, skc2 * 2:skc2 * 2 + 1], scale=0.125)
            st_["pt"].append(pt_t)

        def emit_ctx_chunk(t, skc):
            st_ = state[t]
            if skc == 0:
                st_["psc"] = psC.tile([128, 512], f32, tag="psC", name="psc")
            nc.tensor.matmul(st_["psc"][0:DH + 1, :], lhsT=st_["va"][:, skc, :],
                             rhs=st_["pt"][skc // 2][:, skc % 2, :],
                             start=(skc == 0), stop=(skc == SKC - 1))

        def emit_recip(t):
            st_ = state[t]
            recf = nrm.tile([65, 512], f32, tag="recf", name="recf")
            nc.vector.reciprocal(recf[64:65, :], st_["psc"][64:65, :])
            rec = nrm.tile([65, 512], f32r, tag="rec", name="rec")
            with nc.allow_low_precision(reason="softmax recip rounded to f32r"):
                nc.vector.tensor_copy(rec[64:65, :], recf[64:65, :])
            st_["rec"] = rec

        def emit_norm(t):
            st_ = state[t]
            h, sqb = st_["h"], st_["sqb"]
            psb = psX.tile([128, 512], f32, tag="aux", name="psb")
            nc.tensor.matmul(psb[0:64, :], lhsT=ones[64:65, 0:64],
                             rhs=st_["rec"][64:65, :], start=True, stop=True)
            bc = nrm.tile([64, 512], f32, tag="bc", name="bc")
            nc.vector.tensor_copy(bc, psb[0:64, :])
            cn = cns.tile([64, 512], f32r, tag="cn", name="cn")
            with nc.allow_low_precision(reason="ctx rounded to f32r for out proj"):
                nc.vector.tensor_mul(out=cn, in0=st_["psc"][0:DH, :], in1=bc)
                nc.vector.tensor_scalar_add(cn, cn, bv_sb[0:DH, h:h + 1])
            nc.sync.dma_start(cnd[h * 64:(h + 1) * 64, sqb * 512:(sqb + 1) * 512], cn)
            del state[t]

        cur_kt = cur_va = cur_qt = None
        for t in range(NT):
            h, sqb = divmod(t, 2)
            base = (h % 2) * 64
            st_ = state[t] = {"h": h, "sqb": sqb, "base": base, "pt": []}
            if sqb == 0:
                cur_kt = kts.tile([128, S], f32r, tag="kt", name="kt")
                nc.sync.dma_start(cur_kt[base:base + 64, :], ktd[h * 64:(h + 1) * 64, :])
                cur_va = vas.tile([128, SKC, DH + 1], f32r, tag="va", name="va")
                nc.sync.dma_start(cur_va, vad[:, :, h, :].rearrange("sc p c -> p sc c"))
            st_["kt"], st_["va"] = cur_kt, cur_va
            if sqb == 0:
                cur_qt = qts.tile([128, 2, 512], f32r, tag="qt", name="qt")
                nc.sync.dma_start(
                    cur_qt[base:base + 64, :, :],
                    qtd[h * 64:(h + 1) * 64, :].rearrange("p (a b) -> p a b", a=2))
            st_["qt"] = cur_qt[:, sqb, :]

            for skc2 in range(SKC // 2):
                emit_logits_pair(t, skc2)
                if t >= 1:
                    emit_ctx_chunk(t - 1, skc2 * 2)
                    emit_ctx_chunk(t - 1, skc2 * 2 + 1)
                if t >= 2 and skc2 == 4:
                    emit_norm(t - 2)
            if t >= 1 and t - 1 in state:
                emit_recip(t - 1)

        # pipeline drain
        for skc in range(SKC):
            emit_ctx_chunk(NT - 1, skc)
        emit_norm(NT - 2)
        emit_recip(NT - 1)
        emit_norm(NT - 1)

        # ================= phase C: output projection =================
        wo_sb = wpool.tile([128, KC, D], f32r, tag="w", name="wo_sb")
        nc.sync.dma_start(wo_sb, wo.rearrange("(ko p) d -> p ko d", p=128))
        for st8 in range(8):
            cT = cns2.tile([128, KC, 128], f32r, tag="cT", name="cT")
            nc.sync.dma_start(cT, cnd[:, st8 * 128:(st8 + 1) * 128]
                              .rearrange("(ko p) s -> p ko s", p=128))
            for dh2 in range(2):
                ps = psA.tile([128, 512], f32, tag="psA", name="ps")
                for kc in range(KC):
                    nc.tensor.matmul(ps, lhsT=cT[:, kc, :],
                                     rhs=wo_sb[:, kc, dh2 * 512:(dh2 + 1) * 512],
                                     start=(kc == 0), stop=False)
                nc.tensor.matmul(ps, lhsT=ones[0:1, 0:128],
                                 rhs=bo_sb[0:1, dh2 * 512:(dh2 + 1) * 512],
                                 start=False, stop=True)
                st_t = stg.tile([128, 512], f32, tag="ost", name="ost")
                nc.vector.tensor_copy(st_t, ps)
                nc.sync.dma_start(out[st8 * 128:(st8 + 1) * 128,
                                      dh2 * 512:(dh2 + 1) * 512], st_t)


_NC_CACHE = None


def kernel(query, key, value, mask, wq, bq, wk, bk, wv, bv, wo, bo):
    global _NC_CACHE
    if _NC_CACHE is None:
        _NC_CACHE = _build()
    nc = _NC_CACHE

    query = np.ascontiguousarray(np.asarray(query, dtype=np.float32))
    key = np.ascontiguousarray(np.asarray(key, dtype=np.float32))
    value = np.ascontiguousarray(np.asarray(value, dtype=np.float32))
    mask = np.asarray(mask, dtype=np.float32)

    shared = {
        "wq": np.asarray(wq, np.float32), "wk": np.asarray(wk, np.float32),
        "wv": np.asarray(wv, np.float32), "wo": np.asarray(wo, np.float32),
        "bq": np.asarray(bq, np.float32).reshape(1, D),
        "bk": np.asarray(bk, np.float32).reshape(1, D),
        "bo": np.asarray(bo, np.float32).reshape(1, D),
        "bvT": np.ascontiguousarray(np.asarray(bv, np.float32).reshape(H, DH).T),
        "idn": np.eye(128, dtype=np.float32),
        "one": np.ones((128, 512), np.float32),
    }
    in_maps = []
    for core in range(N_CORES):
        b, half = divmod(core, 2)
        mbc = np.ascontiguousarray(
            (mask[b, 0, 0] * np.float32(-1e9)).reshape(S // 128, 128).T)
        in_maps.append({
            "xq": np.ascontiguousarray(query[b, half * SQ:(half + 1) * SQ]),
            "xk": key[b], "xv": value[b], "mb": mbc, **shared,
        })

    res = run_bass_kernel_spmd(nc, in_maps, core_ids=list(range(N_CORES)))
    full = np.empty((B, S, D), np.float32)
    for core in range(N_CORES):
        b, half = divmod(core, 2)
        full[b, half * SQ:(half + 1) * SQ] = res.results[core]["out"]
    return full


# revision 11
# speedup vs baseline: 1.5046x; 1.2609x over previous
"""MultiHeadAttention Trainium2 kernel.

Full inputs -> full output. Sharding: 8 cores = (batch b in 0..3) x (query
half in 0..1). Each core computes attention for its 1024 query rows of batch
b over all 2048 keys of batch b (K/V projections duplicated across the pair
of cores sharing a batch), then applies the output projection for its rows.
Outputs are disjoint row-slices of (B, S, D); host assembly is a pure concat.

All matmuls run in float32r (fp32 data at ~1 cycle/row, ~2^-12 rounding).

  phase A: transpose x row-blocks via PE-identity transposes, project
           Q^T [d,s], K^T [d,s], V [s,d] (V augmented with a ones column so
           the softmax denominator falls out of the ctx matmul); weight
           matrices stream per 128-row chunk; rank-1 matmuls add bq/bk.
  phase B: software-pipelined (head, query-block) iterations: logits^T
           [sk,sq] = K_h^T.T @ Q_h^T, P = exp(0.125*logits + mask*(-1e9))
           on the scalar engine, interleaved on the PE with the previous
           iteration's ctx matmuls; raw ctx/denominator rows go to DRAM.
  phase C: one batched reciprocal over all 32 denominator rows, PE
           broadcast of the recips, normalize raw ctx^T, then
           out = ctx @ wo + bo' where bo' = bo + bv @ wo (host-folded).
"""

import numpy as np

import concourse.bass as bass
import concourse.mybir as mybir
import concourse.tile as tile
from concourse import bacc
from concourse.bass_utils import run_bass_kernel_spmd

f32 = mybir.dt.float32
f32r = mybir.dt.float32r

B, S, D, H, DH = 4, 2048, 1024, 16, 64
SQ = S // 2          # query rows per core
N_CORES = 8
Exp = mybir.ActivationFunctionType.Exp

KC = D // 128        # 8 contraction chunks
SKC = S // 128       # 16 key chunks
NT = 2 * H           # 32 pipelined iterations (head, sqb)


def _build():
    nc = bacc.Bacc(None, target_bir_lowering=False)

    xq = nc.dram_tensor("xq", [SQ, D], f32r, kind="ExternalInput")
    xk = nc.dram_tensor("xk", [S, D], f32r, kind="ExternalInput")
    xv = nc.dram_tensor("xv", [S, D], f32r, kind="ExternalInput")
    wq = nc.dram_tensor("wq", [D, D], f32r, kind="ExternalInput")
    wk = nc.dram_tensor("wk", [D, D], f32r, kind="ExternalInput")
    wv = nc.dram_tensor("wv", [D, D], f32r, kind="ExternalInput")
    wo = nc.dram_tensor("wo", [D, D], f32r, kind="ExternalInput")
    b3 = nc.dram_tensor("b3", [65, D], f32r, kind="ExternalInput")    # bq/bk/bo' at rows 0/32/64
    mb = nc.dram_tensor("mb", [128, SKC], f32, kind="ExternalInput")  # mask*-1e9 [p, chunk]
    idn = nc.dram_tensor("idn", [128, 128], f32r, kind="ExternalInput")
    one = nc.dram_tensor("one", [128, 512], f32r, kind="ExternalInput")
    sel = nc.dram_tensor("sel", [NT, 2, KC, 128], f32r, kind="ExternalInput")
    out = nc.dram_tensor("out", [SQ, D], f32, kind="ExternalOutput")

    with tile.TileContext(nc) as tc:
        _emit(nc, tc, xq, xk, xv, wq, wk, wv, wo, b3, mb, idn, one, sel, out)
    nc.finalize()
    return nc


def _emit(nc, tc, xq, xk, xv, wq, wk, wv, wo, b3, mb, idn, one, sel, out):
    from contextlib import ExitStack

    with ExitStack() as ctx:
        consts = ctx.enter_context(tc.tile_pool(name="consts", bufs=1))
        wpool = ctx.enter_context(tc.tile_pool(name="wpool", bufs=9))
        xload = ctx.enter_context(tc.tile_pool(name="xload", bufs=2))
        xtp = ctx.enter_context(tc.tile_pool(name="xtp", bufs=1))
        qts = ctx.enter_context(tc.tile_pool(name="qts", bufs=2))
        kts = ctx.enter_context(tc.tile_pool(name="kts", bufs=2))
        vas = ctx.enter_context(tc.tile_pool(name="vas", bufs=2))
        ptp = ctx.enter_context(tc.tile_pool(name="ptp", bufs=11))
        stg = ctx.enter_context(tc.tile_pool(name="stg", bufs=3))
        rbp = ctx.enter_context(tc.tile_pool(name="rbp", bufs=1))
        cns2 = ctx.enter_context(tc.tile_pool(name="cns2", bufs=2))
        psA = ctx.enter_context(tc.tile_pool(name="psA", bufs=2, space="PSUM"))
        psX = ctx.enter_context(tc.tile_pool(name="psX", bufs=2, space="PSUM"))
        psC = ctx.enter_context(tc.tile_pool(name="psC", bufs=2, space="PSUM"))
        dram = ctx.enter_context(tc.tile_pool(name="dram", bufs=1, space="DRAM"))

        ktd = dram.tile([D, S], f32r)                 # K^T
        vad = dram.tile([SKC, 128, H, DH + 1], f32r)  # V augmented with ones col
        qtd = dram.tile([D, SQ], f32r)                # Q^T
        crd = dram.tile([D, SQ], f32r)                # raw (unnormalized) ctx^T
        dnd = dram.tile([NT, 512], f32r)              # denominator rows

        ident = consts.tile([128, 128], f32r)
        nc.sync.dma_start(ident, idn[:])
        ones = consts.tile([128, 512], f32r)
        nc.sync.dma_start(ones, one[:])
        b3_sb = consts.tile([65, D], f32r)
        nc.sync.dma_start(b3_sb, b3[:])
        mb_sb = consts.tile([128, SKC], f32)
        nc.sync.dma_start(mb_sb, mb[:])
        sel_sb = consts.tile([NT, 2, KC, 128], f32r)
        nc.sync.dma_start(sel_sb, sel[:])

        # ---- helper: transpose a [128, 512] slice of a row-block into xT
        def transpose_block(xrow, kc0, xT, col0):
            pst = psX.tile([128, 512], f32r, tag="aux", name="pst")
            for j in range(4):
                nc.tensor.matmul(
                    pst[:, j * 128:(j + 1) * 128],
                    lhsT=xrow[:, (kc0 + j) * 128:(kc0 + j + 1) * 128],
                    rhs=ident[:], is_transpose=True, start=True, stop=True,
                )
            nc.vector.tensor_copy(
                xT[:, kc0:kc0 + 4, col0:col0 + 128],
                pst.rearrange("p (j c) -> p j c", j=4),
            )

        def load_w(w_dram):
            chunks = []
            for kc in range(KC):
                wt = wpool.tile([128, D], f32r, tag="w", name="wt")
                nc.sync.dma_start(wt, w_dram[kc * 128:(kc + 1) * 128, :])
                chunks.append(wt)
            return chunks

        # ================= phase A =================
        def project_T(w_ch, brow, x_dram, blk, dst_dram):
            """One 512-row block of a transposed projection -> dst_dram."""
            xT = xtp.tile([128, KC, 512], f32r, tag="xT", name="xT")
            for st in range(4):
                xrow = xload.tile([128, D], f32r, tag="xr", name="xrow")
                nc.sync.dma_start(xrow, x_dram[(blk * 4 + st) * 128:(blk * 4 + st + 1) * 128, :])
                transpose_block(xrow, 0, xT, st * 128)
                transpose_block(xrow, 4, xT, st * 128)
            for dc2 in range(KC // 2):
                ps = psA.tile([128, 1024], f32, tag="psA", name="ps")
                for half in range(2):
                    dc = dc2 * 2 + half
                    ph = ps[:, half * 512:(half + 1) * 512]
                    for kc in range(KC):
                        nc.tensor.matmul(ph, lhsT=w_ch[kc][:, dc * 128:(dc + 1) * 128],
                                         rhs=xT[:, kc, :], start=(kc == 0), stop=False)
                    nc.tensor.matmul(ph, lhsT=b3_sb[brow:brow + 1, dc * 128:(dc + 1) * 128],
                                     rhs=ones[brow:brow + 1, 0:512], start=False, stop=True)
                st_t = stg.tile([128, 1024], f32r, tag="stg", name="st_t")
                nc.vector.tensor_copy(st_t, ps)
                for half in range(2):
                    dc = dc2 * 2 + half
                    nc.sync.dma_start(
                        dst_dram[dc * 128:(dc + 1) * 128, blk * 512:(blk + 1) * 512],
                        st_t[:, half * 512:(half + 1) * 512])

        wq_ch = load_w(wq)
        for sqb in range(2):
            project_T(wq_ch, 0, xq, sqb, qtd)

        wk_ch = load_w(wk)
        for skb in range(4):
            project_T(wk_ch, 32, xk, skb, ktd)

        wv_ch = load_w(wv)
        for sc in range(SKC):
            nc.sync.dma_start(vad[sc, :, :, DH], ones[:, 0:H])
        for sc in range(SKC):
            xT = xtp.tile([128, KC, 512], f32r, tag="xT", name="xT")
            xrow = xload.tile([128, D], f32r, tag="xr", name="xrow")
            nc.sync.dma_start(xrow, xv[sc * 128:(sc + 1) * 128, :])
            transpose_block(xrow, 0, xT, 0)
            transpose_block(xrow, 4, xT, 0)
            ps = psA.tile([128, 1024], f32, tag="psA", name="ps")
            for dh2 in range(2):
                ph = ps[:, dh2 * 512:(dh2 + 1) * 512]
                for kc in range(KC):
                    nc.tensor.matmul(ph, lhsT=xT[:, kc, 0:128],
                                     rhs=wv_ch[kc][:, dh2 * 512:(dh2 + 1) * 512],
                                     start=(kc == 0), stop=(kc == KC - 1))
            st_t = stg.tile([128, 1024], f32r, tag="stg", name="st_t")
            nc.vector.tensor_copy(st_t, ps)
            nc.sync.dma_start(
                vad[sc, :, :, 0:DH],
                st_t.rearrange("p (h d) -> p h d", h=16),
            )

        # ================= phase B: software-pipelined attention =========
        state = {}

        def emit_logits_pair(t, skc2):
            st_ = state[t]
            psl = psA.tile([128, 1024], f32, tag="psA", name="psl")
            for half in range(2):
                skc = skc2 * 2 + half
                nc.tensor.matmul(psl[:, half * 512:(half + 1) * 512],
                                 lhsT=st_["kt"][st_["base"]:st_["base"] + 64,
                                                skc * 128:(skc + 1) * 128],
                                 rhs=st_["qt"][st_["base"]:st_["base"] + 64, :],
                                 start=True, stop=True)
            pt_t = ptp.tile([128, 2, 512], f32r, tag="pt", name="pt_t")
            nc.scalar.activation(
                pt_t.rearrange("p a b -> p (a b)"), psl, Exp,
                bias=mb_sb[:, skc2 * 2:skc2 * 2 + 1], scale=0.125)
            st_["pt"].append(pt_t)

        def emit_ctx_chunk(t, skc):
            st_ = state[t]
            if skc == 0:
                st_["psc"] = psC.tile([128, 512], f32, tag="psC", name="psc")
            nc.tensor.matmul(st_["psc"][0:DH + 1, :], lhsT=st_["va"][:, skc, :],
                             rhs=st_["pt"][skc // 2][:, skc % 2, :],
                             start=(skc == 0), stop=(skc == SKC - 1))

        def emit_store(t):
            st_ = state[t]
            h, sqb = st_["h"], st_["sqb"]
            cu = stg.tile([65, 512], f32r, tag="cu", name="cu")
            with nc.allow_low_precision(reason="raw ctx rounded to f32r"):
                nc.vector.tensor_copy(cu, st_["psc"][0:DH + 1, :])
            nc.sync.dma_start(crd[h * 64:(h + 1) * 64, sqb * 512:(sqb + 1) * 512],
                              cu[0:DH, :])
            nc.sync.dma_start(dnd[t:t + 1, :], cu[DH:DH + 1, :])
            del state[t]

        cur_kt = cur_va = None
        for t in range(NT):
            h, sqb = divmod(t, 2)
            base = (h % 2) * 64
            st_ = state[t] = {"h": h, "sqb": sqb, "base": base, "pt": []}
            if sqb == 0:
                cur_kt = kts.tile([128, S], f32r, tag="kt", name="kt")
                nc.sync.dma_start(cur_kt[base:base + 64, :], ktd[h * 64:(h + 1) * 64, :])
                cur_va = vas.tile([128, SKC, DH + 1], f32r, tag="va", name="va")
                nc.sync.dma_start(cur_va, vad[:, :, h, :].rearrange("sc p c -> p sc c"))
            st_["kt"], st_["va"] = cur_kt, cur_va
            qt = qts.tile([128, 512], f32r, tag="qt", name="qt")
            nc.sync.dma_start(qt[base:base + 64, :],
                              qtd[h * 64:(h + 1) * 64, sqb * 512:(sqb + 1) * 512])
            st_["qt"] = qt

            for skc2 in range(SKC // 2):
                emit_logits_pair(t, skc2)
                if t >= 1:
                    emit_ctx_chunk(t - 1, skc2 * 2)
                    emit_ctx_chunk(t - 1, skc2 * 2 + 1)
            if t >= 1:
                emit_store(t - 1)

        for skc in range(SKC):
            emit_ctx_chunk(NT - 1, skc)
        emit_store(NT - 1)

        # ================= phase C: normalize + output projection =========
        wo_ch = load_w(wo)
        den_sb = consts.tile([NT, 512], f32r)
        nc.sync.dma_start(den_sb, dnd[:])
        recf = consts.tile([NT, 512], f32)
        nc.vector.reciprocal(recf, den_sb)
        rec = consts.tile([NT, 512], f32r)
        with nc.allow_low_precision(reason="softmax recip rounded to f32r"):
            nc.vector.tensor_copy(rec, recf)

        for sqb in range(2):
            rb = rbp.tile([128, KC, 512], f32r, tag="rb", name="rb")
            for kc in range(KC):
                pb = psX.tile([128, 512], f32, tag="aux", name="pb")
                nc.tensor.matmul(pb, lhsT=sel_sb[:, sqb, kc, :], rhs=rec[:],
                                 start=True, stop=True)
                with nc.allow_low_precision(reason="recip bcast rounded to f32r"):
                    nc.vector.tensor_copy(rb[:, kc, :], pb)
            for st4 in range(4):
                st8 = sqb * 4 + st4
                cT = cns2.tile([128, KC, 128], f32r, tag="cT", name="cT")
                nc.sync.dma_start(cT, crd[:, st8 * 128:(st8 + 1) * 128]
                                  .rearrange("(ko p) s -> p ko s", p=128))
                with nc.allow_low_precision(reason="normalized ctx in f32r"):
                    nc.vector.tensor_mul(out=cT, in0=cT,
                                         in1=rb[:, :, st4 * 128:(st4 + 1) * 128])
                ps = psA.tile([128, 1024], f32, tag="psA", name="ps")
                for dh2 in range(2):
                    ph = ps[:, dh2 * 512:(dh2 + 1) * 512]
                    for kc in range(KC):
                        nc.tensor.matmul(ph, lhsT=cT[:, kc, :],
                                         rhs=wo_ch[kc][:, dh2 * 512:(dh2 + 1) * 512],
                                         start=(kc == 0), stop=False)
                    nc.tensor.matmul(ph, lhsT=ones[64:65, 0:128],
                                     rhs=b3_sb[64:65, dh2 * 512:(dh2 + 1) * 512],
                                     start=False, stop=True)
                st_t = stg.tile([128, 1024], f32, tag="ost", name="ost")
                nc.vector.tensor_copy(st_t, ps)
                nc.sync.dma_start(out[st8 * 128:(st8 + 1) * 128, :], st_t)


_NC_CACHE = None


def _selector():
    s = np.zeros((NT, 2, KC, 128), np.float32)
    for kc in range(KC):
        for p in range(128):
            h = 2 * kc + p // 64
            for sqb in range(2):
                s[2 * h + sqb, sqb, kc, p] = 1.0
    return s


def kernel(query, key, value, mask, wq, bq, wk, bk, wv, bv, wo, bo):
    global _NC_CACHE
    if _NC_CACHE is None:
        _NC_CACHE = _build()
    nc = _NC_CACHE

    query = np.ascontiguousarray(np.asarray(query, dtype=np.float32))
    key = np.ascontiguousarray(np.asarray(key, dtype=np.float32))
    value = np.ascontiguousarray(np.asarray(value, dtype=np.float32))
    mask = np.asarray(mask, dtype=np.float32)
    wo_np = np.asarray(wo, np.float32)
    # fold the V bias through the output projection: (ctx + bv) @ wo + bo
    bo_eff = (np.asarray(bo, np.float64) +
              np.asarray(bv, np.float64) @ np.asarray(wo_np, np.float64)
              ).astype(np.float32)
    b3_host = np.zeros((65, D), np.float32)
    b3_host[0] = np.asarray(bq, np.float32)
    b3_host[32] = np.asarray(bk, np.float32)
    b3_host[64] = bo_eff

    shared = {
        "wq": np.asarray(wq, np.float32), "wk": np.asarray(wk, np.float32),
        "wv": np.asarray(wv, np.float32), "wo": wo_np,
        "b3": b3_host,
        "idn": np.eye(128, dtype=np.float32),
        "one": np.ones((128, 512), np.float32),
        "sel": _selector(),
    }
    in_maps = []
    for core in range(N_CORES):
        b, half = divmod(core, 2)
        mbc = np.ascontiguousarray(
            (mask[b, 0, 0] * np.float32(-1e9)).reshape(S // 128, 128).T)
        in_maps.append({
            "xq": np.ascontiguousarray(query[b, half * SQ:(half + 1) * SQ]),
            "xk": key[b], "xv": value[b], "mb": mbc, **shared,
        })

    res = run_bass_kernel_spmd(nc, in_maps, core_ids=list(range(N_CORES)))
    full = np.empty((B, S, D), np.float32)
    for core in range(N_CORES):
        b, half = divmod(core, 2)
        full[b, half * SQ:(half + 1) * SQ] = res.results[core]["out"]
    return full
